# revision 19
# baseline (speedup 1.0000x reference)
"""MoE routing layer on 8 Trainium2 NeuronCores (data-parallel over batch).

Per core (4 samples):
  routing MLP -> cosine sim vs (host-normalized) embeddings -> softmax
  weights wf[4,10]; w_eff[b] = sum_n wf[b,n] * conv_w[n]; out[b] =
  conv2d(x[b], w_eff[b]) + b_eff[b].

Conv is 9 shifted fp16 matmuls over the flat 58-wide grid; FOUR 64x64
PE quadrants stream concurrently (~196ns per warm tap round).

v5 notes (from v1-v4 traces + microbenchmarks):
 - DMA: per-queue BW ~95GB/s and ~45ns/row generation; a descriptor's
   completion sem fires only when ALL its packets land. blob16 rides as
   two 64-row descriptors (fast), cwp as four [64-row x expert-group]
   descriptors so the DVE chain (experts 0-4) starts ~2us before the
   ACT products (5-9) need their data. x is four 64-row per-sample
   descriptors, pair-1 behind cwp.
 - ALL ACT funcs used are in one table set (natural_log_exp_and_others):
   1/|r| = exp(-0.5*ln(nsq)) instead of Sqrt+reciprocal. Each func-SET
   switch costs a 1.28us table load (v4 paid 4 of them).
 - The weight mix uses UNNORMALIZED exp weights (exbc); the softmax
   1/sum lands in the PSUM drains (ACT scale / DVE dual-scalar TS), so
   softmax-sum -> reciprocal -> wf leaves the critical path.
 - Mix per pair: DVE mul+4xSTT (818ns each) over experts 0-4, ACT
   products (866ns) for 5-9 in parallel, DVE TT folds (545ns). GpSimd
   unused: it shares DVE's SBUF port (measured 2.2x mutual slowdown).
 - exbc-gated PE warmups keep HAM at 2.4GHz into the conv; conv would
   otherwise run its first 3.4us at 1.2GHz.
"""
import sys

sys.path.insert(0, "/opt/trn_rl_repo")

import numpy as np

import concourse.bass as bass
import concourse.mybir as mybir
from concourse.tile import TileContext

F32 = mybir.dt.float32
F16 = mybir.dt.float16
AF = mybir.ActivationFunctionType
ALU = mybir.AluOpType
AX = mybir.AxisListType

NCORES = 8
BLOC = 4           # samples per core
CIN = 64
COUT = 64
H = W = 58
HW = H * W         # 3364
OH = OW = 56
NB = 10            # experts
EDIM = 64
RSIZE = 512
HID = 128
NTAP = 9
CHUNK = 8          # output rows per chunk
NCH = 7            # 7*8 = 56 output rows
NFREE = CHUNK * W  # 464 <= 512 (one PSUM bank)
TAP_OFF = [dy * W + dx for dy in range(3) for dx in range(3)]
NWARM = 13         # exbc-gated PE warmups filling the weight-mix window
ECOLS = NTAP * COUT  # 576 cols per expert
NSPL = 5           # experts 0:NSPL on the DVE chain, rest on ACT

# blob16 column layout (fp16, [128, NCOL16])
C_W1 = 0                 # 512 cols: w1 as [128, 4, 128]
C_W2 = C_W1 + 512        # 64 cols
C_RVT = C_W2 + 64        # 16 cols: rvT as [128, 4, 4]
C_EMBN = C_RVT + 16      # 10 cols: normalized emb.T fp16 (rows 0:64)
C_EXT = C_EMBN + 10      # 12 fp16 cols = 6 f32 cols bitcast
NCOL16 = C_EXT + 12
# f32 view of the EXT block: [128, 6]
# col 0 = b1; col 1 rows 0:64 = b2; cols 2:6 rows 0:4 = eye(4)

# blob10 f32 [NB, NCOL10] layout
B10_CB = 0       # 0:64 cbA, 64:128 cbB
B10_SELA = 128   # selA [4, 128]
B10_SELB = 256
B10_U = 384      # upper-half ones [4, 128]
B10_L = 512      # lower-half ones
B10_MU = 640     # maskU [4, 6]
B10_ML = 646     # maskL [4, 6]
NCOL10 = 652


def fix_sync_waits(nc, cap=2):
    """This walrus build allows at most `cap` sem waits per instruction.
    Splice same-engine NoOps carrying the excess waits right before any
    over-subscribed instruction (waits happen earlier => same semantics)."""
    uid = [0]
    for f in nc.m.functions:
        for blk in f.blocks:
            insts = blk.instructions  # live list
            i = 0
            while i < len(insts):
                inst = insts[i]
                si = inst.sync_info
                waits = list(si.on_wait) if si and si.on_wait else []
                icap = 1
                if len(waits) <= icap:
                    i += 1
                    continue
                keep, excess = waits[-icap:], waits[:-icap]
                for k in range(0, len(excess), icap):
                    nop = mybir.InstNoOp(
                        name=f"{inst.name}-wsplit{uid[0]}", ins=[], outs=[]
                    )
                    uid[0] += 1
                    nop.engine = inst.engine
                    nop.sync_info = mybir.SyncInfo(
                        on_wait=excess[k : k + icap], on_update=[]
                    )
                    nc.register_instruction(nop, overwrite=True)
                    insts.insert(i, nop)
                    i += 1
                inst.sync_info = mybir.SyncInfo(
                    on_wait=keep,
                    on_update=list(si.on_update) if si and si.on_update else [],
                )
                i += 1


def build():
    nc = bass.Bass(num_swdge_queues=1)
    x_d = nc.dram_tensor("x", [BLOC, CIN, HW], F16, kind="ExternalInput")
    cwp_d = nc.dram_tensor("cwp", [128, NB, ECOLS], F16, kind="ExternalInput")
    blob16_d = nc.dram_tensor("blob16", [128, NCOL16], F16, kind="ExternalInput")
    blob10_d = nc.dram_tensor("blob10", [NB, NCOL10], F32, kind="ExternalInput")
    out_d = nc.dram_tensor("out", [BLOC, COUT, OH, OW], F16, kind="ExternalOutput")

    with TileContext(nc) as tc:
        with (
            tc.tile_pool(name="consts", bufs=1) as consts,
            tc.tile_pool(name="work", bufs=2) as work,
            tc.tile_pool(name="stage", bufs=3) as stpool,
            tc.tile_pool(name="ps", bufs=2, space="PSUM") as pspool,
            tc.tile_pool(name="psconv", bufs=2, space="PSUM") as psconv,
            tc.tile_pool(name="pswarm", bufs=1, space="PSUM") as pswarm,
        ):
            # ---------- SBUF constants ----------
            ones64 = consts.tile([EDIM, 1], F16, tag="ones64")
            nc.vector.memset(ones64[:], 1.0)
            blob16 = consts.tile([128, NCOL16], F16, tag="blob16")
            blob10 = consts.tile([NB, NCOL10], F32, tag="blob10")
            cwp2 = consts.tile([128, NB, ECOLS], F16, tag="cwp2")
            xdum = consts.tile([128, NFREE], F16, tag="xdum")
            nc.vector.memset(xdum[:], 0.25)
            xt = []
            for j in range(2):
                t = consts.tile([128, HW + 4], F16, tag=f"xt{j}")
                nc.vector.memset(t[:, HW : HW + 4], 0.0)
                xt.append(t)

            # ---------- DMA dispatch ----------
            nc.sync.dma_start(out=blob16[0:64], in_=blob16_d[0:64])
            nc.sync.dma_start(out=blob10[:], in_=blob10_d[:])
            nc.sync.dma_start(out=cwp2[0:64, 0:NSPL], in_=cwp_d[0:64, 0:NSPL])
            nc.sync.dma_start(out=cwp2[0:64, NSPL:NB], in_=cwp_d[0:64, NSPL:NB])
            nc.scalar.dma_start(out=blob16[64:128], in_=blob16_d[64:128])
            nc.scalar.dma_start(
                out=cwp2[64:128, 0:NSPL], in_=cwp_d[64:128, 0:NSPL]
            )
            nc.scalar.dma_start(
                out=cwp2[64:128, NSPL:NB], in_=cwp_d[64:128, NSPL:NB]
            )
            # x rides the same engine pool as cwp, and engines interleave
            # outstanding descriptors' packets — letting x start immediately
            # stretches cwp completion ~3us (it gates the whole mix). A tiny
            # read of each xt corner, gated on the last cwp slab, makes the
            # x DMA wait (WAR) until cwp is done.
            xgate = work.tile([128, 1], F16, tag="xgate")
            for tile, half in (
                (xt[0], 0), (xt[0], 64), (xt[1], 0), (xt[1], 64)
            ):
                nc.vector.memset(tile[half : half + 1, 0:1], 0.0)
                nc.vector.tensor_scalar_mul(
                    out=xgate[half : half + 1],
                    in0=tile[half : half + 1, 0:1],
                    scalar1=cwp2[half : half + 1, NB - 1, 0:2].bitcast(F32),
                )
            nc.sync.dma_start(out=xt[0][0:64, 0:HW], in_=x_d[0])
            nc.sync.dma_start(out=xt[1][0:64, 0:HW], in_=x_d[2])
            nc.scalar.dma_start(out=xt[0][64:128, 0:HW], in_=x_d[1])
            nc.scalar.dma_start(out=xt[1][64:128, 0:HW], in_=x_d[3])

            w1v = blob16[:, C_W1 : C_W1 + 512].rearrange("p (c m) -> p c m", c=4)
            w2v = blob16[:, C_W2 : C_W2 + 64]
            rvTv = blob16[:, C_RVT : C_RVT + 16].rearrange("p (c b) -> p c b", c=4)
            embnTv = blob16[0:EDIM, C_EMBN : C_EMBN + NB]
            ext = blob16[:, C_EXT : C_EXT + 12].bitcast(F32)
            b1v = ext[:, 0:1]
            b2v = ext[0:EDIM, 1:2]
            id4 = ext[0:4, 2:6]
            cbA = blob10[:, B10_CB : B10_CB + 64]
            cbB = blob10[:, B10_CB + 64 : B10_CB + 128]
            selA = blob10[0:4, B10_SELA : B10_SELA + 128]
            selB = blob10[0:4, B10_SELB : B10_SELB + 128]
            selU = blob10[0:4, B10_U : B10_U + 128]
            selL = blob10[0:4, B10_L : B10_L + 128]
            maskU = blob10[0:4, B10_MU : B10_MU + 6]
            maskL = blob10[0:4, B10_ML : B10_ML + 6]

            # ---------- ACT table prime (one set: ln/exp/relu/identity) ----
            tp_in = work.tile([BLOC, NB], F32, tag="tp_in")
            nc.vector.memset(tp_in[:], 1.0)
            tp_sc = work.tile([BLOC, 1], F32, tag="tp_sc")
            nc.vector.memset(tp_sc[:], 1.0)
            tprime = work.tile([BLOC, NB], F32, tag="tprime")
            tpacc = work.tile([BLOC, 1], F32, tag="tpacc")
            nc.scalar.activation(out=tprime[:, 0:1], in_=tp_sc[:], func=AF.Ln)
            nc.scalar.activation(
                out=tprime[:], in_=tp_in[:], func=AF.Exp,
                scale=tp_sc[:], accum_out=tpacc[:],
            )

            # ---------- routing MLP (fp16 weights) ----------
            h1 = pspool.tile([HID, BLOC], F32, tag="small")
            for c in range(4):
                nc.tensor.matmul(
                    h1[:], w1v[:, c, :], rvTv[:, c, :], start=(c == 0), stop=(c == 3)
                )
            h1r = work.tile([HID, BLOC], F16, tag="h1r")
            nc.scalar.activation(
                out=h1r[:], in_=h1[:], func=AF.Relu, bias=b1v, scale=1.0
            )
            rps = pspool.tile([EDIM, BLOC], F32, tag="small")
            nc.tensor.matmul(rps[:], w2v, h1r[:], start=True, stop=True)
            rsb = work.tile([EDIM, BLOC], F16, tag="rsb")
            nc.scalar.activation(
                out=rsb[:], in_=rps[:], func=AF.Identity, bias=b2v, scale=1.0
            )

            # ---------- r norm + cosine sim + softmax numerators ----------
            rsq = work.tile([EDIM, BLOC], F16, tag="rsq")
            nc.vector.tensor_mul(rsq[:], rsb[:], rsb[:])
            nsq = pspool.tile([BLOC, 1], F32, tag="small")
            nc.tensor.matmul(nsq[:], rsq[:], ones64[:], start=True, stop=True)
            # 1/|r| = exp(-0.5 * ln(|r|^2)) — keeps every ACT in one table set
            lnn = work.tile([BLOC, 1], F32, tag="lnn")
            nc.scalar.activation(out=lnn[:], in_=nsq[:], func=AF.Ln)
            rinv = work.tile([BLOC, 1], F32, tag="rinv")
            nc.scalar.activation(out=rinv[:], in_=lnn[:], func=AF.Exp, scale=-0.5)

            simps = pspool.tile([BLOC, NB], F32, tag="small")
            nc.tensor.matmul(simps[:], rsb[:], embnTv, start=True, stop=True)
            # |cosine| <= 1 so exp() is safe without max subtraction
            ex = work.tile([BLOC, NB], F32, tag="ex")
            s = work.tile([BLOC, 1], F32, tag="s")
            nc.scalar.activation(
                out=ex[:], in_=simps[:], func=AF.Exp, scale=rinv[:], accum_out=s[:]
            )

            # ---------- per-partition broadcast of UNNORMALIZED weights ----
            exbc = []
            for j, sel in enumerate((selA, selB)):
                ps = pspool.tile([128, NB], F32, tag="small")
                nc.tensor.matmul(ps[:], sel, ex[:], start=True, stop=True)
                t = consts.tile([128, NB], F32, tag=f"exbc{j}")
                nc.vector.tensor_scalar_mul(out=t[:], in0=ps[:], scalar1=1.0)
                exbc.append(t)

            # softmax denominator path (off the critical path: used by the
            # PSUM drains and bias matmuls only)
            sinv = work.tile([BLOC, 1], F32, tag="sinv")
            nc.vector.reciprocal(sinv[:], s[:])
            wf = work.tile([BLOC, NB], F32, tag="wf")
            nc.vector.tensor_scalar_mul(out=wf[:], in0=ex[:], scalar1=sinv[:])
            wfT_ps = pspool.tile([NB, BLOC], F32, tag="small")
            nc.tensor.transpose(wfT_ps[:], wf[:], id4)
            wfT = work.tile([NB, BLOC], F32, tag="wfT")
            nc.scalar.copy(out=wfT[:], in_=wfT_ps[:])

            # sinvM[128, 6]: per-partition 1/sum for each drain column
            rhsU = work.tile([BLOC, 6], F32, tag="rhsU")
            nc.vector.tensor_scalar_mul(out=rhsU[:], in0=maskU, scalar1=sinv[:])
            rhsL = work.tile([BLOC, 6], F32, tag="rhsL")
            nc.vector.tensor_scalar_mul(out=rhsL[:], in0=maskL, scalar1=sinv[:])
            sinvM_ps = pspool.tile([128, 6], F32, tag="small")
            nc.tensor.matmul(sinvM_ps[:], selU, rhsU[:], start=True, stop=False)
            nc.tensor.matmul(sinvM_ps[:], selL, rhsL[:], start=False, stop=True)
            sinvM = consts.tile([128, 6], F32, tag="sinvM")
            nc.vector.tensor_scalar_mul(out=sinvM[:], in0=sinvM_ps[:], scalar1=1.0)

            # ---------- drain biases biasM[128, 6] (normalized wf) ----------
            # col: 0=[s0|s2] 1=[s1|s3] 2=[s2|s2] 3=[s3|s3] 4=[s0|s0] 5=[s1|s1]
            bps = pspool.tile([128, 6], F32, tag="small")
            nc.tensor.matmul(
                bps[0:64, 0:4], cbA, wfT[:, 0:4], start=True, stop=True,
                tile_position=(0, 0),
            )
            nc.tensor.matmul(
                bps[0:64, 4:6], cbA, wfT[:, 0:2], start=True, stop=True,
                tile_position=(0, 0),
            )
            nc.tensor.matmul(
                bps[64:128, 0:2], cbB, wfT[:, 2:4], start=True, stop=True,
                tile_position=(0, 64),
            )
            nc.tensor.matmul(
                bps[64:128, 2:4], cbB, wfT[:, 2:4], start=True, stop=True,
                tile_position=(0, 64),
            )
            nc.tensor.matmul(
                bps[64:128, 4:6], cbB, wfT[:, 0:2], start=True, stop=True,
                tile_position=(0, 64),
            )
            biasM = consts.tile([128, 6], F32, tag="biasM")
            nc.scalar.copy(out=biasM[:], in_=bps[:])

            # ---------- PE warmup (gated on exbc via lhsT) ----------
            warm_ps = pswarm.tile([20, NFREE], F32, tag="warm")
            for _ in range(NWARM):
                nc.tensor.matmul(
                    warm_ps[:], exbc[0][:].bitcast(F16), xdum[:],
                    start=True, stop=True,
                )

            # ---------- effective conv weights (unnormalized) ----------
            weff = []
            for j in range(2):
                scrV = consts.tile([128, NTAP, COUT], F16, tag=f"scrV{j}")
                parV = consts.tile([128, NTAP, COUT], F16, tag=f"parV{j}")
                pp = [scrV, parV]
                for k in range(NSPL):
                    if k == 0:
                        nc.vector.tensor_scalar_mul(
                            out=pp[0][:], in0=cwp2[:, 0],
                            scalar1=exbc[j][:, 0:1],
                        )
                    else:
                        nc.vector.scalar_tensor_tensor(
                            out=pp[k % 2][:],
                            in0=cwp2[:, k],
                            scalar=exbc[j][:, k : k + 1],
                            in1=pp[(k - 1) % 2][:],
                            op0=ALU.mult,
                            op1=ALU.add,
                        )
                endV = pp[(NSPL - 1) % 2]
                tmp = []
                for n in range(NSPL, NB):
                    tt = consts.tile([128, NTAP, COUT], F16, tag=f"tmp{j}_{n}")
                    nc.scalar.mul(
                        out=tt[:], in_=cwp2[:, n], mul=exbc[j][:, n : n + 1]
                    )
                    tmp.append(tt)
                g = consts.tile([128, NTAP, COUT], F16, tag=f"g0_{j}")
                nc.vector.tensor_tensor(
                    out=g[:], in0=tmp[0][:], in1=tmp[1][:], op=ALU.add
                )
                for k in range(2, len(tmp)):
                    g2 = consts.tile([128, NTAP, COUT], F16, tag=f"g{k - 1}_{j}")
                    nc.vector.tensor_tensor(
                        out=g2[:], in0=g[:], in1=tmp[k][:], op=ALU.add
                    )
                    g = g2
                t = consts.tile([128, NTAP, COUT], F16, tag=f"weff{j}")
                nc.vector.tensor_tensor(
                    out=t[:], in0=endV[:], in1=g[:], op=ALU.add
                )
                weff.append(t)

            # ---------- conv: 7 groups x 9 taps x 4 quadrants ----------
            # group = (X tile idx, chunk A, Y tile idx, chunk B, biasA, biasB)
            groups = [
                (0, 0, 0, 1, 4, 5),
                (0, 2, 0, 3, 4, 5),
                (0, 4, 0, 5, 4, 5),
                (0, 6, 1, 0, 0, 1),
                (1, 1, 1, 2, 2, 3),
                (1, 3, 1, 4, 2, 3),
                (1, 5, 1, 6, 2, 3),
            ]
            wv = [w[:].rearrange("p t c -> p (t c)") for w in weff]
            for gi, (jx, chA, jy, chB, bcA, bcB) in enumerate(groups):
                wX, wY = wv[jx], wv[jy]
                xX, xY = xt[jx], xt[jy]
                psA = psconv.tile([128, NFREE], F32, tag="psA")
                psB = psconv.tile([128, NFREE], F32, tag="psB")
                for t in range(NTAP):
                    offA = chA * CHUNK * W + TAP_OFF[t]
                    offB = chB * CHUNK * W + TAP_OFF[t]
                    st, sp = (t == 0), (t == NTAP - 1)
                    tc0 = t * COUT
                    nc.tensor.matmul(
                        psA[0:64], wX[0:64, tc0 : tc0 + COUT],
                        xX[0:64, offA : offA + NFREE],
                        start=st, stop=sp, tile_position=(0, 0),
                    )
                    nc.tensor.matmul(
                        psB[0:64], wX[64:128, tc0 : tc0 + COUT],
                        xX[64:128, offA : offA + NFREE],
                        start=st, stop=sp, tile_position=(64, 0),
                    )
                    nc.tensor.matmul(
                        psA[64:128], wY[0:64, tc0 : tc0 + COUT],
                        xY[0:64, offB : offB + NFREE],
                        start=st, stop=sp, tile_position=(0, 64),
                    )
                    nc.tensor.matmul(
                        psB[64:128], wY[64:128, tc0 : tc0 + COUT],
                        xY[64:128, offB : offB + NFREE],
                        start=st, stop=sp, tile_position=(64, 64),
                    )
                # drains apply out = psum * sinv + bias; psA on ACT, psB on
                # ACT while DVE still runs chain B, then on DVE
                stage = stpool.tile([128, 2, CHUNK, OW], F16, tag="st")
                psAv = psA[:].rearrange("p (r w) -> p r w", w=W)[:, :, 0:OW]
                psBv = psB[:].rearrange("p (r w) -> p r w", w=W)[:, :, 0:OW]
                nc.scalar.activation(
                    out=stage[:, 0], in_=psAv, func=AF.Identity,
                    bias=biasM[:, bcA : bcA + 1], scale=sinvM[:, bcA : bcA + 1],
                )
                if gi < 3:
                    nc.scalar.activation(
                        out=stage[:, 1], in_=psBv, func=AF.Identity,
                        bias=biasM[:, bcB : bcB + 1],
                        scale=sinvM[:, bcB : bcB + 1],
                    )
                else:
                    nc.vector.tensor_scalar(
                        out=stage[:, 1], in0=psBv,
                        scalar1=sinvM[:, bcB : bcB + 1],
                        scalar2=biasM[:, bcB : bcB + 1],
                        op0=ALU.mult, op1=ALU.add,
                    )
                # out DMA: one 4D descriptor per partition-half, queues split
                sX0 = 2 * jx  # sample of X half0 (s0 or s2)
                sY0 = 2 * jy
                oA = out_d[sX0 : sX0 + 2, :, chA * CHUNK : chA * CHUNK + CHUNK, :]
                oB = out_d[sY0 : sY0 + 2, :, chB * CHUNK : chB * CHUNK + CHUNK, :]
                nc.sync.dma_start(
                    out=oA.rearrange("s c r w -> c s r w"), in_=stage[0:64]
                )
                # dispatch oB from sync for even groups: the scalar (ACT)
                # engine is near-saturated with drains during the conv
                eng = nc.sync if gi % 2 == 0 else nc.scalar
                eng.dma_start(
                    out=oB.rearrange("s c r w -> c s r w"), in_=stage[64:128]
                )

            warm_sink = work.tile([1, 1], F32, tag="warm_sink")
            nc.scalar.copy(out=warm_sink[:], in_=warm_ps[0:1, 0:1])

    fix_sync_waits(nc)
    return nc


_NC = None


def _get_nc():
    global _NC
    if _NC is None:
        _NC = build()
    return _NC


def make_in_maps(inputs):
    x = np.asarray(inputs["x"], dtype=np.float32)
    rvec = np.asarray(inputs["routing_vector"], dtype=np.float32)
    W1 = np.asarray(inputs["W1"], dtype=np.float32)
    b1 = np.asarray(inputs["b1"], dtype=np.float32)
    W2 = np.asarray(inputs["W2"], dtype=np.float32)
    b2 = np.asarray(inputs["b2"], dtype=np.float32)
    emb = np.asarray(inputs["emb"], dtype=np.float32)
    conv_w = np.asarray(inputs["conv_w"], dtype=np.float32)
    conv_b = np.asarray(inputs["conv_b"], dtype=np.float32)

    x16 = np.ascontiguousarray(
        x.reshape(NCORES, BLOC, CIN, HW).astype(np.float16)
    )
    # conv_w[n, co, ci, ky, kx] -> [128 = ci dup, n, 9*64] partition-major
    cw = conv_w.transpose(0, 2, 3, 4, 1).reshape(NB, CIN, ECOLS)
    cwp = np.ascontiguousarray(
        np.concatenate([cw, cw], axis=1).transpose(1, 0, 2)
    ).astype(np.float16)

    embn = emb / (np.linalg.norm(emb, axis=-1, keepdims=True) + 1e-8)

    blob = np.zeros((128, NCOL16), np.float16)
    blob[:, C_W1 : C_W1 + 512] = (
        W1.reshape(4, 128, HID).transpose(1, 0, 2).reshape(128, 512)
    ).astype(np.float16)
    blob[:, C_W2 : C_W2 + 64] = W2.astype(np.float16)
    blob[0:EDIM, C_EMBN : C_EMBN + NB] = embn.T.astype(np.float16)
    ext = np.zeros((128, 6), np.float32)
    ext[:, 0] = b1
    ext[0:EDIM, 1] = b2
    ext[0:4, 2:6] = np.eye(4, dtype=np.float32)
    blob[:, C_EXT : C_EXT + 12] = ext.view(np.float16)

    blob10 = np.zeros((NB, NCOL10), np.float32)
    blob10[:, 0:64] = conv_b
    blob10[:, 64:128] = conv_b
    sel = np.zeros((2, 4, 128), np.float32)
    for j in range(2):
        sel[j, 2 * j, 0:64] = 1.0
        sel[j, 2 * j + 1, 64:128] = 1.0
    blob10[0:4, B10_SELA : B10_SELA + 128] = sel[0]
    blob10[0:4, B10_SELB : B10_SELB + 128] = sel[1]
    blob10[0:4, B10_U : B10_U + 64] = 1.0
    blob10[0:4, B10_L + 64 : B10_L + 128] = 1.0
    u_map = [0, 1, 2, 3, 0, 1]
    v_map = [2, 3, 2, 3, 0, 1]
    for c in range(6):
        blob10[u_map[c], B10_MU + c] = 1.0
        blob10[v_map[c], B10_ML + c] = 1.0

    in_maps = []
    for c in range(NCORES):
        bc = blob.copy()
        rvc = rvec[BLOC * c : BLOC * (c + 1)]  # [4, 512]
        bc[:, C_RVT : C_RVT + 16] = (
            rvc.T.reshape(4, 128, BLOC).transpose(1, 0, 2).reshape(128, 16)
        ).astype(np.float16)
        in_maps.append(
            {
                "x": x16[c],
                "cwp": cwp,
                "blob16": np.ascontiguousarray(bc),
                "blob10": blob10,
            }
        )
    return in_maps


def kernel(**inputs):
    from concourse.bass_utils import run_bass_kernel_spmd

    nc = _get_nc()
    in_maps = make_in_maps(inputs)
    res = run_bass_kernel_spmd(nc, in_maps, core_ids=list(range(NCORES)))
    return np.concatenate(
        [r["out"].astype(np.float32) for r in res.results], axis=0
    )


# revision 21
# speedup vs baseline: 1.0325x; 1.0325x over previous
"""MoE routing layer on 8 Trainium2 NeuronCores (data-parallel over batch).

Per core (4 samples):
  routing MLP -> cosine sim vs (host-normalized) embeddings -> softmax
  weights wf[4,10]; w_eff[b] = sum_n wf[b,n] * conv_w[n]; out[b] =
  conv2d(x[b], w_eff[b]) + b_eff[b].

Conv is 9 shifted fp16 matmuls over the flat 58-wide grid; FOUR 64x64
PE quadrants stream concurrently (~196ns per warm tap round).

v5 notes (from v1-v4 traces + microbenchmarks):
 - DMA: per-queue BW ~95GB/s and ~45ns/row generation; a descriptor's
   completion sem fires only when ALL its packets land. blob16 rides as
   two 64-row descriptors (fast), cwp as four [64-row x expert-group]
   descriptors so the DVE chain (experts 0-4) starts ~2us before the
   ACT products (5-9) need their data. x is four 64-row per-sample
   descriptors, pair-1 behind cwp.
 - ALL ACT funcs used are in one table set (natural_log_exp_and_others):
   1/|r| = exp(-0.5*ln(nsq)) instead of Sqrt+reciprocal. Each func-SET
   switch costs a 1.28us table load (v4 paid 4 of them).
 - The weight mix uses UNNORMALIZED exp weights (exbc); the softmax
   1/sum lands in the PSUM drains (ACT scale / DVE dual-scalar TS), so
   softmax-sum -> reciprocal -> wf leaves the critical path.
 - Mix per pair: DVE mul+4xSTT (818ns each) over experts 0-4, ACT
   products (866ns) for 5-9 in parallel, DVE TT folds (545ns). GpSimd
   unused: it shares DVE's SBUF port (measured 2.2x mutual slowdown).
 - exbc-gated PE warmups keep HAM at 2.4GHz into the conv; conv would
   otherwise run its first 3.4us at 1.2GHz.
"""
import sys

sys.path.insert(0, "/opt/trn_rl_repo")

import numpy as np

import concourse.bass as bass
import concourse.mybir as mybir
from concourse.tile import TileContext

F32 = mybir.dt.float32
F16 = mybir.dt.float16
AF = mybir.ActivationFunctionType
ALU = mybir.AluOpType
AX = mybir.AxisListType

NCORES = 8
BLOC = 4           # samples per core
CIN = 64
COUT = 64
H = W = 58
HW = H * W         # 3364
OH = OW = 56
NB = 10            # experts
EDIM = 64
RSIZE = 512
HID = 128
NTAP = 9
CHUNK = 8          # output rows per chunk
NCH = 7            # 7*8 = 56 output rows
NFREE = CHUNK * W  # 464 <= 512 (one PSUM bank)
TAP_OFF = [dy * W + dx for dy in range(3) for dx in range(3)]
NWARM = 13         # exbc-gated PE warmups filling the weight-mix window
ECOLS = NTAP * COUT  # 576 cols per expert
NSPL = 7           # experts 0:NSPL on the DVE chain, rest on ACT

# blob16 column layout (fp16, [128, NCOL16])
C_W1 = 0                 # 512 cols: w1 as [128, 4, 128]
C_W2 = C_W1 + 512        # 64 cols
C_RVT = C_W2 + 64        # 16 cols: rvT as [128, 4, 4]
C_EMBN = C_RVT + 16      # 10 cols: normalized emb.T fp16 (rows 0:64)
C_EXT = C_EMBN + 10      # 12 fp16 cols = 6 f32 cols bitcast
NCOL16 = C_EXT + 12
# f32 view of the EXT block: [128, 6]
# col 0 = b1; col 1 rows 0:64 = b2; cols 2:6 rows 0:4 = eye(4)

# blob10 f32 [NB, NCOL10] layout
B10_CB = 0       # 0:64 cbA, 64:128 cbB
B10_SELA = 128   # selA [4, 128]
B10_SELB = 256
B10_U = 384      # upper-half ones [4, 128]
B10_L = 512      # lower-half ones
B10_MU = 640     # maskU [4, 6]
B10_ML = 646     # maskL [4, 6]
NCOL10 = 652


def fix_sync_waits(nc, cap=2):
    """This walrus build allows at most `cap` sem waits per instruction.
    Splice same-engine NoOps carrying the excess waits right before any
    over-subscribed instruction (waits happen earlier => same semantics)."""
    uid = [0]
    for f in nc.m.functions:
        for blk in f.blocks:
            insts = blk.instructions  # live list
            i = 0
            while i < len(insts):
                inst = insts[i]
                si = inst.sync_info
                waits = list(si.on_wait) if si and si.on_wait else []
                icap = 1
                if len(waits) <= icap:
                    i += 1
                    continue
                keep, excess = waits[-icap:], waits[:-icap]
                for k in range(0, len(excess), icap):
                    nop = mybir.InstNoOp(
                        name=f"{inst.name}-wsplit{uid[0]}", ins=[], outs=[]
                    )
                    uid[0] += 1
                    nop.engine = inst.engine
                    nop.sync_info = mybir.SyncInfo(
                        on_wait=excess[k : k + icap], on_update=[]
                    )
                    nc.register_instruction(nop, overwrite=True)
                    insts.insert(i, nop)
                    i += 1
                inst.sync_info = mybir.SyncInfo(
                    on_wait=keep,
                    on_update=list(si.on_update) if si and si.on_update else [],
                )
                i += 1


def build():
    nc = bass.Bass(num_swdge_queues=1)
    x_d = nc.dram_tensor("x", [BLOC, CIN, HW], F16, kind="ExternalInput")
    cwp_d = nc.dram_tensor("cwp", [128, NB, ECOLS], F16, kind="ExternalInput")
    blob16_d = nc.dram_tensor("blob16", [128, NCOL16], F16, kind="ExternalInput")
    blob10_d = nc.dram_tensor("blob10", [NB, NCOL10], F32, kind="ExternalInput")
    out_d = nc.dram_tensor("out", [BLOC, COUT, OH, OW], F16, kind="ExternalOutput")

    with TileContext(nc) as tc:
        with (
            tc.tile_pool(name="consts", bufs=1) as consts,
            tc.tile_pool(name="work", bufs=2) as work,
            tc.tile_pool(name="stage", bufs=3) as stpool,
            tc.tile_pool(name="ps", bufs=2, space="PSUM") as pspool,
            tc.tile_pool(name="psconv", bufs=2, space="PSUM") as psconv,
            tc.tile_pool(name="pswarm", bufs=1, space="PSUM") as pswarm,
        ):
            # ---------- SBUF constants ----------
            ones64 = consts.tile([EDIM, 1], F16, tag="ones64")
            nc.vector.memset(ones64[:], 1.0)
            blob16 = consts.tile([128, NCOL16], F16, tag="blob16")
            blob10 = consts.tile([NB, NCOL10], F32, tag="blob10")
            cwp2 = consts.tile([128, NB, ECOLS], F16, tag="cwp2")
            xdum = consts.tile([128, NFREE], F16, tag="xdum")
            nc.vector.memset(xdum[:], 0.25)
            xt = []
            for j in range(2):
                t = consts.tile([128, HW + 4], F16, tag=f"xt{j}")
                nc.vector.memset(t[:, HW : HW + 4], 0.0)
                xt.append(t)

            # ---------- DMA dispatch ----------
            nc.sync.dma_start(out=blob16[0:64], in_=blob16_d[0:64])
            nc.sync.dma_start(out=blob10[:], in_=blob10_d[:])
            nc.sync.dma_start(out=cwp2[0:64, 0:NSPL], in_=cwp_d[0:64, 0:NSPL])
            nc.sync.dma_start(out=cwp2[0:64, NSPL:NB], in_=cwp_d[0:64, NSPL:NB])
            nc.scalar.dma_start(out=blob16[64:128], in_=blob16_d[64:128])
            nc.scalar.dma_start(
                out=cwp2[64:128, 0:NSPL], in_=cwp_d[64:128, 0:NSPL]
            )
            nc.scalar.dma_start(
                out=cwp2[64:128, NSPL:NB], in_=cwp_d[64:128, NSPL:NB]
            )
            nc.sync.dma_start(out=xt[0][0:64, 0:HW], in_=x_d[0])
            nc.sync.dma_start(out=xt[1][0:64, 0:HW], in_=x_d[2])
            nc.scalar.dma_start(out=xt[0][64:128, 0:HW], in_=x_d[1])
            nc.scalar.dma_start(out=xt[1][64:128, 0:HW], in_=x_d[3])

            w1v = blob16[:, C_W1 : C_W1 + 512].rearrange("p (c m) -> p c m", c=4)
            w2v = blob16[:, C_W2 : C_W2 + 64]
            rvTv = blob16[:, C_RVT : C_RVT + 16].rearrange("p (c b) -> p c b", c=4)
            embnTv = blob16[0:EDIM, C_EMBN : C_EMBN + NB]
            ext = blob16[:, C_EXT : C_EXT + 12].bitcast(F32)
            b1v = ext[:, 0:1]
            b2v = ext[0:EDIM, 1:2]
            id4 = ext[0:4, 2:6]
            cbA = blob10[:, B10_CB : B10_CB + 64]
            cbB = blob10[:, B10_CB + 64 : B10_CB + 128]
            selA = blob10[0:4, B10_SELA : B10_SELA + 128]
            selB = blob10[0:4, B10_SELB : B10_SELB + 128]
            selU = blob10[0:4, B10_U : B10_U + 128]
            selL = blob10[0:4, B10_L : B10_L + 128]
            maskU = blob10[0:4, B10_MU : B10_MU + 6]
            maskL = blob10[0:4, B10_ML : B10_ML + 6]

            # ---------- ACT table prime (one set: ln/exp/relu/identity) ----
            tp_in = work.tile([BLOC, NB], F32, tag="tp_in")
            nc.vector.memset(tp_in[:], 1.0)
            tp_sc = work.tile([BLOC, 1], F32, tag="tp_sc")
            nc.vector.memset(tp_sc[:], 1.0)
            tprime = work.tile([BLOC, NB], F32, tag="tprime")
            tpacc = work.tile([BLOC, 1], F32, tag="tpacc")
            nc.scalar.activation(out=tprime[:, 0:1], in_=tp_sc[:], func=AF.Ln)
            nc.scalar.activation(
                out=tprime[:], in_=tp_in[:], func=AF.Exp,
                scale=tp_sc[:], accum_out=tpacc[:],
            )

            # ---------- routing MLP (fp16 weights) ----------
            h1 = pspool.tile([HID, BLOC], F32, tag="small")
            for c in range(4):
                nc.tensor.matmul(
                    h1[:], w1v[:, c, :], rvTv[:, c, :], start=(c == 0), stop=(c == 3)
                )
            h1r = work.tile([HID, BLOC], F16, tag="h1r")
            nc.scalar.activation(
                out=h1r[:], in_=h1[:], func=AF.Relu, bias=b1v, scale=1.0
            )
            rps = pspool.tile([EDIM, BLOC], F32, tag="small")
            nc.tensor.matmul(rps[:], w2v, h1r[:], start=True, stop=True)
            rsb = work.tile([EDIM, BLOC], F16, tag="rsb")
            nc.scalar.activation(
                out=rsb[:], in_=rps[:], func=AF.Identity, bias=b2v, scale=1.0
            )

            # ---------- r norm + cosine sim + softmax numerators ----------
            rsq = work.tile([EDIM, BLOC], F16, tag="rsq")
            nc.vector.tensor_mul(rsq[:], rsb[:], rsb[:])
            nsq = pspool.tile([BLOC, 1], F32, tag="small")
            nc.tensor.matmul(nsq[:], rsq[:], ones64[:], start=True, stop=True)
            # 1/|r| = exp(-0.5 * ln(|r|^2)) — keeps every ACT in one table set
            lnn = work.tile([BLOC, 1], F32, tag="lnn")
            nc.scalar.activation(out=lnn[:], in_=nsq[:], func=AF.Ln)
            rinv = work.tile([BLOC, 1], F32, tag="rinv")
            nc.scalar.activation(out=rinv[:], in_=lnn[:], func=AF.Exp, scale=-0.5)

            simps = pspool.tile([BLOC, NB], F32, tag="small")
            nc.tensor.matmul(simps[:], rsb[:], embnTv, start=True, stop=True)
            # |cosine| <= 1 so exp() is safe without max subtraction
            ex = work.tile([BLOC, NB], F32, tag="ex")
            s = work.tile([BLOC, 1], F32, tag="s")
            nc.scalar.activation(
                out=ex[:], in_=simps[:], func=AF.Exp, scale=rinv[:], accum_out=s[:]
            )

            # ---------- per-partition broadcast of UNNORMALIZED weights ----
            exbc = []
            for j, sel in enumerate((selA, selB)):
                ps = pspool.tile([128, NB], F32, tag="small")
                nc.tensor.matmul(ps[:], sel, ex[:], start=True, stop=True)
                t = consts.tile([128, NB], F32, tag=f"exbc{j}")
                nc.vector.tensor_scalar_mul(out=t[:], in0=ps[:], scalar1=1.0)
                exbc.append(t)

            # softmax denominator path (off the critical path: used by the
            # PSUM drains and bias matmuls only)
            sinv = work.tile([BLOC, 1], F32, tag="sinv")
            nc.vector.reciprocal(sinv[:], s[:])
            wf = work.tile([BLOC, NB], F32, tag="wf")
            nc.vector.tensor_scalar_mul(out=wf[:], in0=ex[:], scalar1=sinv[:])
            wfT_ps = pspool.tile([NB, BLOC], F32, tag="small")
            nc.tensor.transpose(wfT_ps[:], wf[:], id4)
            wfT = work.tile([NB, BLOC], F32, tag="wfT")
            nc.scalar.copy(out=wfT[:], in_=wfT_ps[:])

            # sinvM[128, 6]: per-partition 1/sum for each drain column
            rhsU = work.tile([BLOC, 6], F32, tag="rhsU")
            nc.vector.tensor_scalar_mul(out=rhsU[:], in0=maskU, scalar1=sinv[:])
            rhsL = work.tile([BLOC, 6], F32, tag="rhsL")
            nc.vector.tensor_scalar_mul(out=rhsL[:], in0=maskL, scalar1=sinv[:])
            sinvM_ps = pspool.tile([128, 6], F32, tag="small")
            nc.tensor.matmul(sinvM_ps[:], selU, rhsU[:], start=True, stop=False)
            nc.tensor.matmul(sinvM_ps[:], selL, rhsL[:], start=False, stop=True)
            sinvM = consts.tile([128, 6], F32, tag="sinvM")
            nc.vector.tensor_scalar_mul(out=sinvM[:], in0=sinvM_ps[:], scalar1=1.0)

            # ---------- drain biases biasM[128, 6] (normalized wf) ----------
            # col: 0=[s0|s2] 1=[s1|s3] 2=[s2|s2] 3=[s3|s3] 4=[s0|s0] 5=[s1|s1]
            bps = pspool.tile([128, 6], F32, tag="small")
            nc.tensor.matmul(
                bps[0:64, 0:4], cbA, wfT[:, 0:4], start=True, stop=True,
                tile_position=(0, 0),
            )
            nc.tensor.matmul(
                bps[0:64, 4:6], cbA, wfT[:, 0:2], start=True, stop=True,
                tile_position=(0, 0),
            )
            nc.tensor.matmul(
                bps[64:128, 0:2], cbB, wfT[:, 2:4], start=True, stop=True,
                tile_position=(0, 64),
            )
            nc.tensor.matmul(
                bps[64:128, 2:4], cbB, wfT[:, 2:4], start=True, stop=True,
                tile_position=(0, 64),
            )
            nc.tensor.matmul(
                bps[64:128, 4:6], cbB, wfT[:, 0:2], start=True, stop=True,
                tile_position=(0, 64),
            )
            biasM = consts.tile([128, 6], F32, tag="biasM")
            nc.scalar.copy(out=biasM[:], in_=bps[:])

            # ---------- PE warmup (gated on exbc via lhsT) ----------
            warm_ps = pswarm.tile([20, NFREE], F32, tag="warm")
            for _ in range(NWARM):
                nc.tensor.matmul(
                    warm_ps[:], exbc[0][:].bitcast(F16), xdum[:],
                    start=True, stop=True,
                )

            # ---------- effective conv weights (unnormalized) ----------
            weff = []
            for j in range(2):
                scrV = consts.tile([128, NTAP, COUT], F16, tag=f"scrV{j}")
                parV = consts.tile([128, NTAP, COUT], F16, tag=f"parV{j}")
                pp = [scrV, parV]
                for k in range(NSPL):
                    if k == 0:
                        nc.vector.tensor_scalar_mul(
                            out=pp[0][:], in0=cwp2[:, 0],
                            scalar1=exbc[j][:, 0:1],
                        )
                    else:
                        nc.vector.scalar_tensor_tensor(
                            out=pp[k % 2][:],
                            in0=cwp2[:, k],
                            scalar=exbc[j][:, k : k + 1],
                            in1=pp[(k - 1) % 2][:],
                            op0=ALU.mult,
                            op1=ALU.add,
                        )
                endV = pp[(NSPL - 1) % 2]
                tmp = []
                for n in range(NSPL, NB):
                    tt = consts.tile([128, NTAP, COUT], F16, tag=f"tmp{j}_{n}")
                    nc.scalar.mul(
                        out=tt[:], in_=cwp2[:, n], mul=exbc[j][:, n : n + 1]
                    )
                    tmp.append(tt)
                g = consts.tile([128, NTAP, COUT], F16, tag=f"g0_{j}")
                nc.vector.tensor_tensor(
                    out=g[:], in0=tmp[0][:], in1=tmp[1][:], op=ALU.add
                )
                for k in range(2, len(tmp)):
                    g2 = consts.tile([128, NTAP, COUT], F16, tag=f"g{k - 1}_{j}")
                    nc.vector.tensor_tensor(
                        out=g2[:], in0=g[:], in1=tmp[k][:], op=ALU.add
                    )
                    g = g2
                t = consts.tile([128, NTAP, COUT], F16, tag=f"weff{j}")
                nc.vector.tensor_tensor(
                    out=t[:], in0=endV[:], in1=g[:], op=ALU.add
                )
                weff.append(t)

            # ---------- conv: 7 groups x 9 taps x 4 quadrants ----------
            # group = (X tile idx, chunk A, Y tile idx, chunk B, biasA, biasB)
            groups = [
                (0, 0, 0, 1, 4, 5),
                (0, 2, 0, 3, 4, 5),
                (0, 4, 0, 5, 4, 5),
                (0, 6, 1, 0, 0, 1),
                (1, 1, 1, 2, 2, 3),
                (1, 3, 1, 4, 2, 3),
                (1, 5, 1, 6, 2, 3),
            ]
            wv = [w[:].rearrange("p t c -> p (t c)") for w in weff]
            for gi, (jx, chA, jy, chB, bcA, bcB) in enumerate(groups):
                wX, wY = wv[jx], wv[jy]
                xX, xY = xt[jx], xt[jy]
                psA = psconv.tile([128, NFREE], F32, tag="psA")
                psB = psconv.tile([128, NFREE], F32, tag="psB")
                for t in range(NTAP):
                    offA = chA * CHUNK * W + TAP_OFF[t]
                    offB = chB * CHUNK * W + TAP_OFF[t]
                    st, sp = (t == 0), (t == NTAP - 1)
                    tc0 = t * COUT
                    nc.tensor.matmul(
                        psA[0:64], wX[0:64, tc0 : tc0 + COUT],
                        xX[0:64, offA : offA + NFREE],
                        start=st, stop=sp, tile_position=(0, 0),
                    )
                    nc.tensor.matmul(
                        psB[0:64], wX[64:128, tc0 : tc0 + COUT],
                        xX[64:128, offA : offA + NFREE],
                        start=st, stop=sp, tile_position=(64, 0),
                    )
                    nc.tensor.matmul(
                        psA[64:128], wY[0:64, tc0 : tc0 + COUT],
                        xY[0:64, offB : offB + NFREE],
                        start=st, stop=sp, tile_position=(0, 64),
                    )
                    nc.tensor.matmul(
                        psB[64:128], wY[64:128, tc0 : tc0 + COUT],
                        xY[64:128, offB : offB + NFREE],
                        start=st, stop=sp, tile_position=(64, 64),
                    )
                # drains apply out = psum * sinv + bias; psA on ACT, psB on
                # ACT while DVE still runs chain B, then on DVE
                stage = stpool.tile([128, 2, CHUNK, OW], F16, tag="st")
                psAv = psA[:].rearrange("p (r w) -> p r w", w=W)[:, :, 0:OW]
                psBv = psB[:].rearrange("p (r w) -> p r w", w=W)[:, :, 0:OW]
                nc.scalar.activation(
                    out=stage[:, 0], in_=psAv, func=AF.Identity,
                    bias=biasM[:, bcA : bcA + 1], scale=sinvM[:, bcA : bcA + 1],
                )
                if gi < 3:
                    nc.scalar.activation(
                        out=stage[:, 1], in_=psBv, func=AF.Identity,
                        bias=biasM[:, bcB : bcB + 1],
                        scale=sinvM[:, bcB : bcB + 1],
                    )
                else:
                    nc.vector.tensor_scalar(
                        out=stage[:, 1], in0=psBv,
                        scalar1=sinvM[:, bcB : bcB + 1],
                        scalar2=biasM[:, bcB : bcB + 1],
                        op0=ALU.mult, op1=ALU.add,
                    )
                # out DMA: one 4D descriptor per partition-half, queues split
                sX0 = 2 * jx  # sample of X half0 (s0 or s2)
                sY0 = 2 * jy
                oA = out_d[sX0 : sX0 + 2, :, chA * CHUNK : chA * CHUNK + CHUNK, :]
                oB = out_d[sY0 : sY0 + 2, :, chB * CHUNK : chB * CHUNK + CHUNK, :]
                nc.sync.dma_start(
                    out=oA.rearrange("s c r w -> c s r w"), in_=stage[0:64]
                )
                # dispatch oB from sync for even groups: the scalar (ACT)
                # engine is near-saturated with drains during the conv
                eng = nc.sync if gi % 2 == 0 else nc.scalar
                eng.dma_start(
                    out=oB.rearrange("s c r w -> c s r w"), in_=stage[64:128]
                )

            warm_sink = work.tile([1, 1], F32, tag="warm_sink")
            nc.scalar.copy(out=warm_sink[:], in_=warm_ps[0:1, 0:1])

    fix_sync_waits(nc)
    return nc


_NC = None


def _get_nc():
    global _NC
    if _NC is None:
        _NC = build()
    return _NC


def make_in_maps(inputs):
    x = np.asarray(inputs["x"], dtype=np.float32)
    rvec = np.asarray(inputs["routing_vector"], dtype=np.float32)
    W1 = np.asarray(inputs["W1"], dtype=np.float32)
    b1 = np.asarray(inputs["b1"], dtype=np.float32)
    W2 = np.asarray(inputs["W2"], dtype=np.float32)
    b2 = np.asarray(inputs["b2"], dtype=np.float32)
    emb = np.asarray(inputs["emb"], dtype=np.float32)
    conv_w = np.asarray(inputs["conv_w"], dtype=np.float32)
    conv_b = np.asarray(inputs["conv_b"], dtype=np.float32)

    x16 = np.ascontiguousarray(
        x.reshape(NCORES, BLOC, CIN, HW).astype(np.float16)
    )
    # conv_w[n, co, ci, ky, kx] -> [128 = ci dup, n, 9*64] partition-major
    cw = conv_w.transpose(0, 2, 3, 4, 1).reshape(NB, CIN, ECOLS)
    cwp = np.ascontiguousarray(
        np.concatenate([cw, cw], axis=1).transpose(1, 0, 2)
    ).astype(np.float16)

    embn = emb / (np.linalg.norm(emb, axis=-1, keepdims=True) + 1e-8)

    blob = np.zeros((128, NCOL16), np.float16)
    blob[:, C_W1 : C_W1 + 512] = (
        W1.reshape(4, 128, HID).transpose(1, 0, 2).reshape(128, 512)
    ).astype(np.float16)
    blob[:, C_W2 : C_W2 + 64] = W2.astype(np.float16)
    blob[0:EDIM, C_EMBN : C_EMBN + NB] = embn.T.astype(np.float16)
    ext = np.zeros((128, 6), np.float32)
    ext[:, 0] = b1
    ext[0:EDIM, 1] = b2
    ext[0:4, 2:6] = np.eye(4, dtype=np.float32)
    blob[:, C_EXT : C_EXT + 12] = ext.view(np.float16)

    blob10 = np.zeros((NB, NCOL10), np.float32)
    blob10[:, 0:64] = conv_b
    blob10[:, 64:128] = conv_b
    sel = np.zeros((2, 4, 128), np.float32)
    for j in range(2):
        sel[j, 2 * j, 0:64] = 1.0
        sel[j, 2 * j + 1, 64:128] = 1.0
    blob10[0:4, B10_SELA : B10_SELA + 128] = sel[0]
    blob10[0:4, B10_SELB : B10_SELB + 128] = sel[1]
    blob10[0:4, B10_U : B10_U + 64] = 1.0
    blob10[0:4, B10_L + 64 : B10_L + 128] = 1.0
    u_map = [0, 1, 2, 3, 0, 1]
    v_map = [2, 3, 2, 3, 0, 1]
    for c in range(6):
        blob10[u_map[c], B10_MU + c] = 1.0
        blob10[v_map[c], B10_ML + c] = 1.0

    in_maps = []
    for c in range(NCORES):
        bc = blob.copy()
        rvc = rvec[BLOC * c : BLOC * (c + 1)]  # [4, 512]
        bc[:, C_RVT : C_RVT + 16] = (
            rvc.T.reshape(4, 128, BLOC).transpose(1, 0, 2).reshape(128, 16)
        ).astype(np.float16)
        in_maps.append(
            {
                "x": x16[c],
                "cwp": cwp,
                "blob16": np.ascontiguousarray(bc),
                "blob10": blob10,
            }
        )
    return in_maps


def kernel(**inputs):
    from concourse.bass_utils import run_bass_kernel_spmd

    nc = _get_nc()
    in_maps = make_in_maps(inputs)
    res = run_bass_kernel_spmd(nc, in_maps, core_ids=list(range(NCORES)))
    return np.concatenate(
        [r["out"].astype(np.float32) for r in res.results], axis=0
    )


# revision 23
# speedup vs baseline: 1.1028x; 1.0680x over previous
"""MoE routing layer on 8 Trainium2 NeuronCores (data-parallel over batch).

Per core (4 samples):
  routing MLP -> cosine sim vs (host-normalized) embeddings -> softmax
  weights wf[4,10]; w_eff[b] = sum_n wf[b,n] * conv_w[n]; out[b] =
  conv2d(x[b], w_eff[b]) + b_eff[b].

Conv is 9 shifted fp16 matmuls over the flat 58-wide grid; FOUR 64x64
PE quadrants stream concurrently (~196ns per warm tap round).

v5 notes (from v1-v4 traces + microbenchmarks):
 - DMA: per-queue BW ~95GB/s and ~45ns/row generation; a descriptor's
   completion sem fires only when ALL its packets land. blob16 rides as
   two 64-row descriptors (fast), cwp as four [64-row x expert-group]
   descriptors so the DVE chain (experts 0-4) starts ~2us before the
   ACT products (5-9) need their data. x is four 64-row per-sample
   descriptors, pair-1 behind cwp.
 - ALL ACT funcs used are in one table set (natural_log_exp_and_others):
   1/|r| = exp(-0.5*ln(nsq)) instead of Sqrt+reciprocal. Each func-SET
   switch costs a 1.28us table load (v4 paid 4 of them).
 - The weight mix uses UNNORMALIZED exp weights (exbc); the softmax
   1/sum lands in the PSUM drains (ACT scale / DVE dual-scalar TS), so
   softmax-sum -> reciprocal -> wf leaves the critical path.
 - Mix per pair: DVE mul+4xSTT (818ns each) over experts 0-4, ACT
   products (866ns) for 5-9 in parallel, DVE TT folds (545ns). GpSimd
   unused: it shares DVE's SBUF port (measured 2.2x mutual slowdown).
 - exbc-gated PE warmups keep HAM at 2.4GHz into the conv; conv would
   otherwise run its first 3.4us at 1.2GHz.
"""
import sys

sys.path.insert(0, "/opt/trn_rl_repo")

import numpy as np

import concourse.bass as bass
import concourse.mybir as mybir
from concourse.tile import TileContext

F32 = mybir.dt.float32
F16 = mybir.dt.float16
AF = mybir.ActivationFunctionType
ALU = mybir.AluOpType
AX = mybir.AxisListType

NCORES = 8
BLOC = 4           # samples per core
CIN = 64
COUT = 64
H = W = 58
HW = H * W         # 3364
OH = OW = 56
NB = 10            # experts
EDIM = 64
RSIZE = 512
HID = 128
NTAP = 9
CHUNK = 8          # output rows per chunk
NCH = 7            # 7*8 = 56 output rows
NFREE = CHUNK * W  # 464 <= 512 (one PSUM bank)
TAP_OFF = [dy * W + dx for dy in range(3) for dx in range(3)]
NWARM = 13         # exbc-gated PE warmups filling the weight-mix window
ECOLS = NTAP * COUT  # 576 cols per expert
NSPL = 5           # experts 0:NSPL on the DVE chain, rest on ACT

# blob16 column layout (fp16, [128, NCOL16])
C_W1 = 0                 # 512 cols: w1 as [128, 4, 128]
C_W2 = C_W1 + 512        # 64 cols
C_RVT = C_W2 + 64        # 16 cols: rvT as [128, 4, 4]
C_EMBN = C_RVT + 16      # 10 cols: normalized emb.T fp16 (rows 0:64)
C_EXT = C_EMBN + 10      # 12 fp16 cols = 6 f32 cols bitcast
NCOL16 = C_EXT + 12
# f32 view of the EXT block: [128, 6]
# col 0 = b1; col 1 rows 0:64 = b2; cols 2:6 rows 0:4 = eye(4)

# blob10 f32 [NB, NCOL10] layout
B10_CB = 0       # 0:64 cbA, 64:128 cbB
B10_SELA = 128   # selA [4, 128]
B10_SELB = 256
B10_U = 384      # upper-half ones [4, 128]
B10_L = 512      # lower-half ones
B10_MU = 640     # maskU [4, 6]
B10_ML = 646     # maskL [4, 6]
NCOL10 = 652


def fix_sync_waits(nc, cap=2):
    """This walrus build allows at most `cap` sem waits per instruction.
    Splice same-engine NoOps carrying the excess waits right before any
    over-subscribed instruction (waits happen earlier => same semantics)."""
    uid = [0]
    for f in nc.m.functions:
        for blk in f.blocks:
            insts = blk.instructions  # live list
            i = 0
            while i < len(insts):
                inst = insts[i]
                si = inst.sync_info
                waits = list(si.on_wait) if si and si.on_wait else []
                icap = 1
                if len(waits) <= icap:
                    i += 1
                    continue
                keep, excess = waits[-icap:], waits[:-icap]
                for k in range(0, len(excess), icap):
                    nop = mybir.InstNoOp(
                        name=f"{inst.name}-wsplit{uid[0]}", ins=[], outs=[]
                    )
                    uid[0] += 1
                    nop.engine = inst.engine
                    nop.sync_info = mybir.SyncInfo(
                        on_wait=excess[k : k + icap], on_update=[]
                    )
                    nc.register_instruction(nop, overwrite=True)
                    insts.insert(i, nop)
                    i += 1
                inst.sync_info = mybir.SyncInfo(
                    on_wait=keep,
                    on_update=list(si.on_update) if si and si.on_update else [],
                )
                i += 1


def build():
    nc = bass.Bass(num_swdge_queues=1)
    x_d = nc.dram_tensor("x", [BLOC, CIN, HW], F16, kind="ExternalInput")
    cwp_d = nc.dram_tensor("cwp", [128, NB, ECOLS], F16, kind="ExternalInput")
    blob16_d = nc.dram_tensor("blob16", [128, NCOL16], F16, kind="ExternalInput")
    blob10_d = nc.dram_tensor("blob10", [NB, NCOL10], F32, kind="ExternalInput")
    out_d = nc.dram_tensor("out", [BLOC, COUT, OH, OW], F16, kind="ExternalOutput")

    with TileContext(nc) as tc:
        with (
            tc.tile_pool(name="consts", bufs=1) as consts,
            tc.tile_pool(name="work", bufs=2) as work,
            tc.tile_pool(name="stage", bufs=3) as stpool,
            tc.tile_pool(name="ps", bufs=2, space="PSUM") as pspool,
            tc.tile_pool(name="psconv", bufs=2, space="PSUM") as psconv,
            tc.tile_pool(name="pswarm", bufs=1, space="PSUM") as pswarm,
        ):
            # ---------- SBUF constants ----------
            ones64 = consts.tile([EDIM, 1], F16, tag="ones64")
            nc.vector.memset(ones64[:], 1.0)
            blob16 = consts.tile([128, NCOL16], F16, tag="blob16")
            blob10 = consts.tile([NB, NCOL10], F32, tag="blob10")
            cwp2 = consts.tile([128, NB, ECOLS], F16, tag="cwp2")
            xdum = consts.tile([128, NFREE], F16, tag="xdum")
            nc.vector.memset(xdum[:], 0.25)
            xt = []
            for j in range(2):
                t = consts.tile([128, HW + 4], F16, tag=f"xt{j}")
                nc.vector.memset(t[:, HW : HW + 4], 0.0)
                xt.append(t)

            # ---------- DMA dispatch ----------
            nc.sync.dma_start(out=blob16[0:64], in_=blob16_d[0:64])
            nc.sync.dma_start(out=blob10[:], in_=blob10_d[:])
            nc.sync.dma_start(out=cwp2[0:64, 0:NSPL], in_=cwp_d[0:64, 0:NSPL])
            nc.sync.dma_start(out=cwp2[0:64, NSPL:NB], in_=cwp_d[0:64, NSPL:NB])
            nc.scalar.dma_start(out=blob16[64:128], in_=blob16_d[64:128])
            nc.scalar.dma_start(
                out=cwp2[64:128, 0:NSPL], in_=cwp_d[64:128, 0:NSPL]
            )
            nc.scalar.dma_start(
                out=cwp2[64:128, NSPL:NB], in_=cwp_d[64:128, NSPL:NB]
            )
            nc.sync.dma_start(out=xt[0][0:64, 0:HW], in_=x_d[0])
            nc.sync.dma_start(out=xt[1][0:64, 0:HW], in_=x_d[2])
            nc.scalar.dma_start(out=xt[0][64:128, 0:HW], in_=x_d[1])
            nc.scalar.dma_start(out=xt[1][64:128, 0:HW], in_=x_d[3])

            w1v = blob16[:, C_W1 : C_W1 + 512].rearrange("p (c m) -> p c m", c=4)
            w2v = blob16[:, C_W2 : C_W2 + 64]
            rvTv = blob16[:, C_RVT : C_RVT + 16].rearrange("p (c b) -> p c b", c=4)
            embnTv = blob16[0:EDIM, C_EMBN : C_EMBN + NB]
            ext = blob16[:, C_EXT : C_EXT + 12].bitcast(F32)
            b1v = ext[:, 0:1]
            b2v = ext[0:EDIM, 1:2]
            id4 = ext[0:4, 2:6]
            cbA = blob10[:, B10_CB : B10_CB + 64]
            cbB = blob10[:, B10_CB + 64 : B10_CB + 128]
            selA = blob10[0:4, B10_SELA : B10_SELA + 128]
            selB = blob10[0:4, B10_SELB : B10_SELB + 128]
            selU = blob10[0:4, B10_U : B10_U + 128]
            selL = blob10[0:4, B10_L : B10_L + 128]
            maskU = blob10[0:4, B10_MU : B10_MU + 6]
            maskL = blob10[0:4, B10_ML : B10_ML + 6]

            # ---------- ACT table prime (one set: ln/exp/relu/identity) ----
            tp_in = work.tile([BLOC, NB], F32, tag="tp_in")
            nc.vector.memset(tp_in[:], 1.0)
            tp_sc = work.tile([BLOC, 1], F32, tag="tp_sc")
            nc.vector.memset(tp_sc[:], 1.0)
            tprime = work.tile([BLOC, NB], F32, tag="tprime")
            tpacc = work.tile([BLOC, 1], F32, tag="tpacc")
            nc.scalar.activation(out=tprime[:, 0:1], in_=tp_sc[:], func=AF.Ln)
            nc.scalar.activation(
                out=tprime[:], in_=tp_in[:], func=AF.Exp,
                scale=tp_sc[:], accum_out=tpacc[:],
            )

            # ---------- routing MLP (fp16 weights) ----------
            h1 = pspool.tile([HID, BLOC], F32, tag="small")
            for c in range(4):
                nc.tensor.matmul(
                    h1[:], w1v[:, c, :], rvTv[:, c, :], start=(c == 0), stop=(c == 3)
                )
            h1r = work.tile([HID, BLOC], F16, tag="h1r")
            nc.scalar.activation(
                out=h1r[:], in_=h1[:], func=AF.Relu, bias=b1v, scale=1.0
            )
            rps = pspool.tile([EDIM, BLOC], F32, tag="small")
            nc.tensor.matmul(rps[:], w2v, h1r[:], start=True, stop=True)
            rsb = work.tile([EDIM, BLOC], F16, tag="rsb")
            nc.scalar.activation(
                out=rsb[:], in_=rps[:], func=AF.Identity, bias=b2v, scale=1.0
            )

            # ---------- r norm + cosine sim + softmax numerators ----------
            rsq = work.tile([EDIM, BLOC], F16, tag="rsq")
            nc.vector.tensor_mul(rsq[:], rsb[:], rsb[:])
            nsq = pspool.tile([BLOC, 1], F32, tag="small")
            nc.tensor.matmul(nsq[:], rsq[:], ones64[:], start=True, stop=True)
            # 1/|r| = exp(-0.5 * ln(|r|^2)) — keeps every ACT in one table set
            lnn = work.tile([BLOC, 1], F32, tag="lnn")
            nc.scalar.activation(out=lnn[:], in_=nsq[:], func=AF.Ln)
            rinv = work.tile([BLOC, 1], F32, tag="rinv")
            nc.scalar.activation(out=rinv[:], in_=lnn[:], func=AF.Exp, scale=-0.5)

            simps = pspool.tile([BLOC, NB], F32, tag="small")
            nc.tensor.matmul(simps[:], rsb[:], embnTv, start=True, stop=True)
            # |cosine| <= 1 so exp() is safe without max subtraction
            ex = work.tile([BLOC, NB], F32, tag="ex")
            s = work.tile([BLOC, 1], F32, tag="s")
            nc.scalar.activation(
                out=ex[:], in_=simps[:], func=AF.Exp, scale=rinv[:], accum_out=s[:]
            )

            # ---------- per-partition broadcast of UNNORMALIZED weights ----
            exbc = []
            for j, sel in enumerate((selA, selB)):
                ps = pspool.tile([128, NB], F32, tag="small")
                nc.tensor.matmul(ps[:], sel, ex[:], start=True, stop=True)
                t = consts.tile([128, NB], F32, tag=f"exbc{j}")
                nc.vector.tensor_scalar_mul(out=t[:], in0=ps[:], scalar1=1.0)
                exbc.append(t)

            # softmax denominator path (off the critical path: used by the
            # PSUM drains and bias matmuls only)
            sinv = work.tile([BLOC, 1], F32, tag="sinv")
            nc.vector.reciprocal(sinv[:], s[:])
            wf = work.tile([BLOC, NB], F32, tag="wf")
            nc.vector.tensor_scalar_mul(out=wf[:], in0=ex[:], scalar1=sinv[:])
            wfT_ps = pspool.tile([NB, BLOC], F32, tag="small")
            nc.tensor.transpose(wfT_ps[:], wf[:], id4)
            wfT = work.tile([NB, BLOC], F32, tag="wfT")
            nc.scalar.copy(out=wfT[:], in_=wfT_ps[:])

            # sinvM[128, 6]: per-partition 1/sum for each drain column
            rhsU = work.tile([BLOC, 6], F32, tag="rhsU")
            nc.vector.tensor_scalar_mul(out=rhsU[:], in0=maskU, scalar1=sinv[:])
            rhsL = work.tile([BLOC, 6], F32, tag="rhsL")
            nc.vector.tensor_scalar_mul(out=rhsL[:], in0=maskL, scalar1=sinv[:])
            sinvM_ps = pspool.tile([128, 6], F32, tag="small")
            nc.tensor.matmul(sinvM_ps[:], selU, rhsU[:], start=True, stop=False)
            nc.tensor.matmul(sinvM_ps[:], selL, rhsL[:], start=False, stop=True)
            sinvM = consts.tile([128, 6], F32, tag="sinvM")
            nc.vector.tensor_scalar_mul(out=sinvM[:], in0=sinvM_ps[:], scalar1=1.0)

            # ---------- drain biases biasM[128, 6] (normalized wf) ----------
            # col: 0=[s0|s2] 1=[s1|s3] 2=[s2|s2] 3=[s3|s3] 4=[s0|s0] 5=[s1|s1]
            bps = pspool.tile([128, 6], F32, tag="small")
            nc.tensor.matmul(
                bps[0:64, 0:4], cbA, wfT[:, 0:4], start=True, stop=True,
                tile_position=(0, 0),
            )
            nc.tensor.matmul(
                bps[0:64, 4:6], cbA, wfT[:, 0:2], start=True, stop=True,
                tile_position=(0, 0),
            )
            nc.tensor.matmul(
                bps[64:128, 0:2], cbB, wfT[:, 2:4], start=True, stop=True,
                tile_position=(0, 64),
            )
            nc.tensor.matmul(
                bps[64:128, 2:4], cbB, wfT[:, 2:4], start=True, stop=True,
                tile_position=(0, 64),
            )
            nc.tensor.matmul(
                bps[64:128, 4:6], cbB, wfT[:, 0:2], start=True, stop=True,
                tile_position=(0, 64),
            )
            biasM = consts.tile([128, 6], F32, tag="biasM")
            nc.scalar.copy(out=biasM[:], in_=bps[:])

            # ---------- PE warmup (gated on exbc via lhsT) ----------
            warm_ps = pswarm.tile([20, NFREE], F32, tag="warm")
            for _ in range(NWARM):
                nc.tensor.matmul(
                    warm_ps[:], exbc[0][:].bitcast(F16), xdum[:],
                    start=True, stop=True,
                )

            # ---------- effective conv weights (unnormalized) ----------
            weff = []
            for j in range(2):
                scrV = consts.tile([128, NTAP, COUT], F16, tag=f"scrV{j}")
                parV = consts.tile([128, NTAP, COUT], F16, tag=f"parV{j}")
                pp = [scrV, parV]
                for k in range(NSPL):
                    if k == 0:
                        nc.vector.tensor_scalar_mul(
                            out=pp[0][:], in0=cwp2[:, 0],
                            scalar1=exbc[j][:, 0:1],
                        )
                    else:
                        nc.vector.scalar_tensor_tensor(
                            out=pp[k % 2][:],
                            in0=cwp2[:, k],
                            scalar=exbc[j][:, k : k + 1],
                            in1=pp[(k - 1) % 2][:],
                            op0=ALU.mult,
                            op1=ALU.add,
                        )
                endV = pp[(NSPL - 1) % 2]
                tmp = []
                for n in range(NSPL, NB):
                    tt = consts.tile([128, NTAP, COUT], F16, tag=f"tmp{j}_{n}")
                    nc.scalar.mul(
                        out=tt[:], in_=cwp2[:, n], mul=exbc[j][:, n : n + 1]
                    )
                    tmp.append(tt)
                g = consts.tile([128, NTAP, COUT], F16, tag=f"g0_{j}")
                nc.vector.tensor_tensor(
                    out=g[:], in0=tmp[0][:], in1=tmp[1][:], op=ALU.add
                )
                for k in range(2, len(tmp)):
                    g2 = consts.tile([128, NTAP, COUT], F16, tag=f"g{k - 1}_{j}")
                    nc.vector.tensor_tensor(
                        out=g2[:], in0=g[:], in1=tmp[k][:], op=ALU.add
                    )
                    g = g2
                t = consts.tile([128, NTAP, COUT], F16, tag=f"weff{j}")
                nc.vector.tensor_tensor(
                    out=t[:], in0=endV[:], in1=g[:], op=ALU.add
                )
                weff.append(t)

            # ---------- conv: 7 groups x 9 taps x 4 quadrants ----------
            # group = (X tile idx, chunk A, Y tile idx, chunk B, biasA, biasB)
            groups = [
                (0, 0, 0, 1, 4, 5),
                (0, 2, 0, 3, 4, 5),
                (0, 4, 0, 5, 4, 5),
                (0, 6, 1, 0, 0, 1),
                (1, 1, 1, 2, 2, 3),
                (1, 3, 1, 4, 2, 3),
                (1, 5, 1, 6, 2, 3),
            ]
            wv = [w[:].rearrange("p t c -> p (t c)") for w in weff]
            for gi, (jx, chA, jy, chB, bcA, bcB) in enumerate(groups):
                wX, wY = wv[jx], wv[jy]
                xX, xY = xt[jx], xt[jy]
                psA = psconv.tile([128, NFREE], F32, tag="psA")
                psB = psconv.tile([128, NFREE], F32, tag="psB")
                for t in range(NTAP):
                    offA = chA * CHUNK * W + TAP_OFF[t]
                    offB = chB * CHUNK * W + TAP_OFF[t]
                    st, sp = (t == 0), (t == NTAP - 1)
                    tc0 = t * COUT
                    nc.tensor.matmul(
                        psA[0:64], wX[0:64, tc0 : tc0 + COUT],
                        xX[0:64, offA : offA + NFREE],
                        start=st, stop=sp, tile_position=(0, 0),
                    )
                    nc.tensor.matmul(
                        psB[0:64], wX[64:128, tc0 : tc0 + COUT],
                        xX[64:128, offA : offA + NFREE],
                        start=st, stop=sp, tile_position=(64, 0),
                    )
                    nc.tensor.matmul(
                        psA[64:128], wY[0:64, tc0 : tc0 + COUT],
                        xY[0:64, offB : offB + NFREE],
                        start=st, stop=sp, tile_position=(0, 64),
                    )
                    nc.tensor.matmul(
                        psB[64:128], wY[64:128, tc0 : tc0 + COUT],
                        xY[64:128, offB : offB + NFREE],
                        start=st, stop=sp, tile_position=(64, 64),
                    )
                # drains apply out = psum * sinv + bias; psA on ACT, psB on
                # ACT while DVE still runs chain B, then on DVE
                stage = stpool.tile([128, 2, CHUNK, OW], F16, tag="st")
                psAv = psA[:].rearrange("p (r w) -> p r w", w=W)[:, :, 0:OW]
                psBv = psB[:].rearrange("p (r w) -> p r w", w=W)[:, :, 0:OW]
                nc.scalar.activation(
                    out=stage[:, 0], in_=psAv, func=AF.Identity,
                    bias=biasM[:, bcA : bcA + 1], scale=sinvM[:, bcA : bcA + 1],
                )
                if gi < 3:
                    nc.scalar.activation(
                        out=stage[:, 1], in_=psBv, func=AF.Identity,
                        bias=biasM[:, bcB : bcB + 1],
                        scale=sinvM[:, bcB : bcB + 1],
                    )
                else:
                    nc.vector.tensor_scalar(
                        out=stage[:, 1], in0=psBv,
                        scalar1=sinvM[:, bcB : bcB + 1],
                        scalar2=biasM[:, bcB : bcB + 1],
                        op0=ALU.mult, op1=ALU.add,
                    )
                # out DMA: one 4D descriptor per partition-half, queues split
                sX0 = 2 * jx  # sample of X half0 (s0 or s2)
                sY0 = 2 * jy
                oA = out_d[sX0 : sX0 + 2, :, chA * CHUNK : chA * CHUNK + CHUNK, :]
                oB = out_d[sY0 : sY0 + 2, :, chB * CHUNK : chB * CHUNK + CHUNK, :]
                nc.sync.dma_start(
                    out=oA.rearrange("s c r w -> c s r w"), in_=stage[0:64]
                )
                nc.scalar.dma_start(
                    out=oB.rearrange("s c r w -> c s r w"), in_=stage[64:128]
                )

            warm_sink = work.tile([1, 1], F32, tag="warm_sink")
            nc.scalar.copy(out=warm_sink[:], in_=warm_ps[0:1, 0:1])

    fix_sync_waits(nc)
    return nc


_NC = None


def _get_nc():
    global _NC
    if _NC is None:
        _NC = build()
    return _NC


def make_in_maps(inputs):
    x = np.asarray(inputs["x"], dtype=np.float32)
    rvec = np.asarray(inputs["routing_vector"], dtype=np.float32)
    W1 = np.asarray(inputs["W1"], dtype=np.float32)
    b1 = np.asarray(inputs["b1"], dtype=np.float32)
    W2 = np.asarray(inputs["W2"], dtype=np.float32)
    b2 = np.asarray(inputs["b2"], dtype=np.float32)
    emb = np.asarray(inputs["emb"], dtype=np.float32)
    conv_w = np.asarray(inputs["conv_w"], dtype=np.float32)
    conv_b = np.asarray(inputs["conv_b"], dtype=np.float32)

    x16 = np.ascontiguousarray(
        x.reshape(NCORES, BLOC, CIN, HW).astype(np.float16)
    )
    # conv_w[n, co, ci, ky, kx] -> [128 = ci dup, n, 9*64] partition-major
    cw = conv_w.transpose(0, 2, 3, 4, 1).reshape(NB, CIN, ECOLS)
    cwp = np.ascontiguousarray(
        np.concatenate([cw, cw], axis=1).transpose(1, 0, 2)
    ).astype(np.float16)

    embn = emb / (np.linalg.norm(emb, axis=-1, keepdims=True) + 1e-8)

    blob = np.zeros((128, NCOL16), np.float16)
    blob[:, C_W1 : C_W1 + 512] = (
        W1.reshape(4, 128, HID).transpose(1, 0, 2).reshape(128, 512)
    ).astype(np.float16)
    blob[:, C_W2 : C_W2 + 64] = W2.astype(np.float16)
    blob[0:EDIM, C_EMBN : C_EMBN + NB] = embn.T.astype(np.float16)
    ext = np.zeros((128, 6), np.float32)
    ext[:, 0] = b1
    ext[0:EDIM, 1] = b2
    ext[0:4, 2:6] = np.eye(4, dtype=np.float32)
    blob[:, C_EXT : C_EXT + 12] = ext.view(np.float16)

    blob10 = np.zeros((NB, NCOL10), np.float32)
    blob10[:, 0:64] = conv_b
    blob10[:, 64:128] = conv_b
    sel = np.zeros((2, 4, 128), np.float32)
    for j in range(2):
        sel[j, 2 * j, 0:64] = 1.0
        sel[j, 2 * j + 1, 64:128] = 1.0
    blob10[0:4, B10_SELA : B10_SELA + 128] = sel[0]
    blob10[0:4, B10_SELB : B10_SELB + 128] = sel[1]
    blob10[0:4, B10_U : B10_U + 64] = 1.0
    blob10[0:4, B10_L + 64 : B10_L + 128] = 1.0
    u_map = [0, 1, 2, 3, 0, 1]
    v_map = [2, 3, 2, 3, 0, 1]
    for c in range(6):
        blob10[u_map[c], B10_MU + c] = 1.0
        blob10[v_map[c], B10_ML + c] = 1.0

    in_maps = []
    for c in range(NCORES):
        bc = blob.copy()
        rvc = rvec[BLOC * c : BLOC * (c + 1)]  # [4, 512]
        bc[:, C_RVT : C_RVT + 16] = (
            rvc.T.reshape(4, 128, BLOC).transpose(1, 0, 2).reshape(128, 16)
        ).astype(np.float16)
        in_maps.append(
            {
                "x": x16[c],
                "cwp": cwp,
                "blob16": np.ascontiguousarray(bc),
                "blob10": blob10,
            }
        )
    return in_maps


def kernel(**inputs):
    from concourse.bass_utils import run_bass_kernel_spmd

    nc = _get_nc()
    in_maps = make_in_maps(inputs)
    res = run_bass_kernel_spmd(nc, in_maps, core_ids=list(range(NCORES)))
    return np.concatenate(
        [r["out"].astype(np.float32) for r in res.results], axis=0
    )
